# revision 1
# baseline (speedup 1.0000x reference)
"""Trainium2 Bass kernel for AttentionConv2d (self-attention over 64x64 pixels).

Reference math (per image b):
    xf = x.reshape(C, N)                      # C=256, N=4096
    q  = Wq @ xf + bq                         # [32, N]
    k  = Wk @ xf + bk                         # [32, N]
    v  = Wv @ xf + bv                         # [256, N]
    corr[i, j] = sum_c q[c, i] * k[c, j]      # [N, N]
    beta = softmax(corr, axis=0)              # over i, per column j
    att[c, j] = gamma * sum_i v[c, i] * beta[i, j]
    out = att.reshape(C, H, W) + x

Sharding: data-parallel over batch, one image per NeuronCore (8 cores).

Per-core design (measured ~198 us vs 288 us for the naive-ordering version):
  - corr matmuls are 4x row-tiled (tile_position=(32r,0)): K=32 uses only a
    quarter of the PE contraction rows, so four i-tiles run concurrently on
    the four 32-row strips (quad span ~320ns vs 4x213ns serial).  q/k are
    produced 4x-replicated across partition groups for free by widening the
    projection weights host-side (np.tile(W.T,(1,4))).
  - projections / v^T run in float32r (full-rate fp32): x is never cast,
    saving a DVE pass; PV runs in bf16 (E tiles and v^T tiles).
  - flat software pipeline over all 64 (j-block, quad) steps with PV lagging
    corr/exp by LAG=8 quads (a full j-block): the Scalar engine's exp stream
    (the per-block floor, 16 x 1.07us) never waits on att evacuation or
    trailing PV work.  j0's corr quads interleave into the input chunk loop.
  - softmax denominator: non-destructive bf16 pair-sum accumulation on DVE
    (PV still needs the raw E a block later), partition-reduce + broadcast
    on the otherwise idle GpSimd engine (no PSUM slot -> never blocks the
    corr quad pipeline); the last block uses a ones-matmul broadcast instead
    for a shorter tail.
  - normalization multiplies att straight out of PSUM (rb is ready before
    PV(j) finishes); att is split into two single-bank tiles in separate
    pools so release is bank-granular -- the next block's h=0 PV chain only
    waits on the h=0 mul (worth ~13us over a fused double-bank att tile).
  - PSUM: 3x[128,1024] eps slots (corr quad outputs, one exp per half) +
    2x[128,512] att accumulators = 8 banks.
  - head: weights land in 3 packed DMAs (dma_start issue costs ~0.6us of
    sequencer time each) and ~4us of dummy matmuls during the preamble warm
    the PE HAM clock gate so the first projections run at 2.4 GHz.
gamma is folded into Wv host-side; gamma*bv is added at the end (softmax
weights sum to 1, so the v-bias is a per-channel constant).
"""

import sys

sys.path.insert(0, "/opt/trn_rl_repo")

from contextlib import ExitStack

import numpy as np

C = 256
CR = 32
N = 4096
CH = 128          # channel half (partition dim)
JB = 512          # j-block width (one PSUM bank of fp32)
NJ = N // JB      # 8 j-blocks
IT = 128          # i-tile height (partition dim of E tiles)
NI = N // IT      # 32 i-tiles
NC = 8            # x column chunks (512 wide)
LAG = 8           # quads of PV lag behind corr/exp (a full j-block:
                  # decouples the Scalar exp stream from att evacuation)


def _build_program():
    import concourse.bass as bass
    import concourse.mybir as mybir
    from concourse import bacc, bass_isa, tile

    f32 = mybir.dt.float32
    f32r = mybir.dt.float32r
    bf16 = mybir.dt.bfloat16
    EXP = mybir.ActivationFunctionType.Exp
    ADD = mybir.AluOpType.add
    ts = bass.ts

    nc = bacc.Bacc()
    x_d = nc.declare_dram_parameter("x", [C, N], f32r, isOutput=False)
    wpack_d = nc.declare_dram_parameter("wpack", [C, 512], f32r, isOutput=False)
    bpack_d = nc.declare_dram_parameter("bpack", [128, 4], f32, isOutput=False)
    out_d = nc.declare_dram_parameter("out", [C, N], f32, isOutput=True)

    with TileCtx(tile, nc) as (tc, ctx):
        const = ctx.enter_context(tc.tile_pool(name="const", bufs=1))
        vtp = ctx.enter_context(tc.tile_pool(name="vtp", bufs=1))
        ebp = ctx.enter_context(tc.tile_pool(name="ebp", bufs=3))
        work = ctx.enter_context(tc.tile_pool(name="work", bufs=2))
        outp = ctx.enter_context(tc.tile_pool(name="outp", bufs=2))
        # PSUM: eps 3x[128,1024] = 6 banks, att 1x[128,1024] = 2 banks
        eps_p = ctx.enter_context(tc.tile_pool(name="eps_p", bufs=3, space="PSUM"))
        # att split into two 1-bank tiles in separate pools: bank-granular
        # release, so the next block's first PV matmul (h=0) only waits on
        # the h=0 normalization mul, not both
        att0_p = ctx.enter_context(tc.tile_pool(name="att0_p", bufs=1, space="PSUM"))
        att1_p = ctx.enter_context(tc.tile_pool(name="att1_p", bufs=1, space="PSUM"))

        # ---- resident weights: 3 packed DMAs (dma_start issue is ~0.6us of
        # sequencer time each, so fewer transfers = earlier first projection)
        wp = []
        for h in range(2):
            t = const.tile([CH, 512], f32r, name=f"wpack{h}")
            nc.scalar.dma_start(out=t[:], in_=wpack_d[h * CH:(h + 1) * CH, :])
            wp.append(t)
        bpack = const.tile([128, 4], f32, name="bpack")
        nc.scalar.dma_start(out=bpack[:], in_=bpack_d[:, :])
        wq4t = [wp[h][:, 0:128] for h in range(2)]
        wk4t = [wp[h][:, 128:256] for h in range(2)]
        wvt = [wp[h][:, 256:512] for h in range(2)]
        bq4_t = bpack[:, 0:1]
        bk4_t = bpack[:, 1:2]
        gbv = [bpack[:, 2 + h:3 + h] for h in range(2)]
        ones_b = const.tile([128, 128], bf16, name="ones_b")
        nc.vector.memset(ones_b[:], 1.0)
        touch = const.tile([CH, 1], f32, name="touch")
        nc.vector.tensor_copy(touch[:], wp[0][:, 0:1].bitcast(f32))
        nc.vector.tensor_copy(touch[:], wp[1][:, 0:1].bitcast(f32))
        nc.vector.tensor_copy(touch[:], bpack[:, 0:1])
        # warm the PE HAM clock gate during the preamble: ~4us of dummy
        # matmuls so the real projections run at 2.4 GHz, not 1.2
        warm = eps_p.tile([128, 1024], f32, tag="eps", name="eps")
        for _ in range(24):
            nc.tensor.matmul(
                warm[:, 0:128], lhsT=ones_b[:], rhs=ones_b[:], start=True, stop=True
            )

        # ---- x / projections / v^T, chunk-pipelined -------------------------
        # j=0's corr quads + exps are interleaved into the chunk loop so the
        # Scalar engine starts the softmax exp stream as early as possible.
        # Projection PSUMs use the att-pool slot (idle until the first PV,
        # ~35us in) so the next chunk's projection never waits for this
        # chunk's exps to free an eps slot.
        xf = [const.tile([CH, N], f32r, name=f"xf{h}") for h in range(2)]
        q4 = const.tile([128, N], bf16, name="q4")
        k4 = const.tile([128, N], bf16, name="k4")
        vt = []

        def corr_quad(eblk, j, g):
            """4x row-tiled S matmuls + exp for quad g (i-tiles 4g..4g+3)."""
            jsl = ts(j, JB)
            epsA = eps_p.tile([128, 1024], f32, tag="eps", name="eps")
            epsB = eps_p.tile([128, 1024], f32, tag="eps", name="eps")
            for r in range(4):
                i = 4 * g + r
                dst = epsA if r < 2 else epsB
                nc.tensor.matmul(
                    dst[:, ts(r % 2, JB)],
                    lhsT=q4[32 * r:32 * (r + 1), ts(i, IT)],
                    rhs=k4[32 * r:32 * (r + 1), jsl],
                    start=True,
                    stop=True,
                    tile_position=(32 * r, 0),
                )
            nc.scalar.activation(eblk[:, ts(2 * g, 1024)], epsA[:], EXP)
            nc.scalar.activation(eblk[:, ts(2 * g + 1, 1024)], epsB[:], EXP)

        eblk0 = ebp.tile([IT, NI * JB], bf16, tag="eblk", name="eblk")
        for c in range(NC):
            csl = ts(c, JB)
            for h in range(2):
                nc.sync.dma_start(out=xf[h][:, csl], in_=x_d[h * CH:(h + 1) * CH, csl])
            for (dst, wt, bias) in ((q4, wq4t, bq4_t), (k4, wk4t, bk4_t)):
                ps = eps_p.tile([128, 1024], f32, tag="eps", name="eps")
                for h in range(2):
                    nc.tensor.matmul(
                        ps[:, 0:JB],
                        lhsT=wt[h],
                        rhs=xf[h][:, csl],
                        start=(h == 0),
                        stop=(h == 1),
                    )
                nc.vector.tensor_scalar_add(dst[:, csl], ps[:, 0:JB], bias)
            corr_quad(eblk0, 0, c)
            psv = eps_p.tile([128, 1024], f32, tag="eps", name="eps")
            for t4 in range(4):
                i = 4 * c + t4
                for h in range(2):
                    nc.tensor.matmul(
                        psv[:, ts(t4, C)],
                        lhsT=xf[h][:, ts(i, IT)],
                        rhs=wvt[h],
                        start=(h == 0),
                        stop=(h == 1),
                    )
            vtile = vtp.tile([128, 1024], bf16, name=f"vt{c}")
            nc.any.tensor_copy(vtile[:], psv[:])
            vt.append(vtile)

        def pv_quad(att2, eblk, g):
            """PV accumulation matmuls for quad g (i-tiles 4g..4g+3)."""
            for t4 in range(4):
                i = 4 * g + t4
                for h in range(2):
                    nc.tensor.matmul(
                        att2[h][:],
                        lhsT=vt[i // 4][:, i % 4 * C + h * CH: i % 4 * C + (h + 1) * CH],
                        rhs=eblk[:, ts(i, JB)],
                        start=(i == 0),
                        stop=(i == NI - 1),
                    )

        # ---- main attention loop: flat software pipeline over all quads ----
        # corr/exp for quad idx runs LAG=8 quads (one j-block) ahead of PV.
        # The denominator chain (pair sums -> partition reduce -> reciprocal)
        # completes before PV(j) finishes, so the output chain reads att
        # straight from PSUM right after PV(j,7) and frees the att bank fast.
        eblks = {0: eblk0}
        atts = {}
        rbs = {}
        accs = {}

        def denom_tail(j):
            acc = accs[j]
            nc.vector.tensor_add(acc[:, 0:JB], acc[:, 0:JB], acc[:, JB:2 * JB])
            s_part = acc[:, 0:JB]
            rb = work.tile([CH, JB], f32, tag="rb", name="rb")
            rscr = work.tile([CH, JB], f32, tag="rscr", name="rscr")
            if j < NJ - 1:
                # partition-reduce + broadcast on the (otherwise idle) GpSimd
                # engine: no PSUM slot, never blocks the corr quad pipeline
                s_bc = work.tile([CH, JB], f32, tag="s_bc", name="s_bc")
                nc.gpsimd.partition_all_reduce(
                    s_bc[:], s_part, channels=CH, reduce_op=bass_isa.ReduceOp.add
                )
                nc.vector.reciprocal_approx_accurate(out=rb[:], in_=s_bc[:], scratch=rscr[:])
            else:
                # last block: ones-matmul broadcast (short latency, and no
                # following block to collide with in the PSUM slot FIFO)
                smm = eps_p.tile([128, 1024], f32, tag="eps", name="eps")
                nc.tensor.matmul(
                    smm[:, 0:JB], lhsT=ones_b[:], rhs=s_part, start=True, stop=True
                )
                nc.vector.reciprocal_approx_accurate(out=rb[:], in_=smm[:, 0:JB], scratch=rscr[:])
            rbs[j] = rb

        def out_tail(j):
            jsl = ts(j, JB)
            # both att-releasing muls first (frees both att banks ASAP for the
            # next block's PV), then the residual adds and DMAs
            os = []
            for h in range(2):
                o = outp.tile([CH, JB], f32, tag="o", name="o")
                nc.vector.tensor_mul(o[:], atts[j][h][:], rbs[j][:])
                os.append(o)
            for h in range(2):
                nc.vector.scalar_tensor_tensor(
                    out=os[h][:],
                    in0=os[h][:],
                    scalar=gbv[h],
                    in1=xf[h][:, jsl].bitcast(f32),
                    op0=ADD,
                    op1=ADD,
                )
                eng = nc.scalar if j == NJ - 1 else nc.sync
                eng.dma_start(out=out_d[h * CH:(h + 1) * CH, jsl], in_=os[h][:])

        QT = NJ * 8
        for idx in range(QT + LAG):
            p = idx - LAG
            if p >= 0:
                p_j, p_g = divmod(p, 8)
                if p_g == 0:
                    atts[p_j] = (
                        att0_p.tile([CH, JB], f32, tag="att0", name="att0"),
                        att1_p.tile([CH, JB], f32, tag="att1", name="att1"),
                    )
                pv_quad(atts[p_j], eblks[p_j], p_g)
                if p_g == 7:
                    out_tail(p_j)
            if idx < QT:
                q_j, q_g = divmod(idx, 8)
                if q_j >= 1:
                    if q_g == 0:
                        eblks[q_j] = ebp.tile([IT, NI * JB], bf16, tag="eblk", name="eblk")
                    corr_quad(eblks[q_j], q_j, q_g)
                # incremental denominator: non-destructive pair sums so PV
                # (a block behind) still sees the raw E values
                eb = eblks[q_j]
                pair = work.tile([128, 1024], bf16, tag="pair", name="pair")
                nc.vector.tensor_add(
                    pair[:], eb[:, ts(2 * q_g, 1024)], eb[:, ts(2 * q_g + 1, 1024)]
                )
                if q_g == 0:
                    accs[q_j] = work.tile([128, 1024], bf16, tag="acc1", name="acc1")
                    nc.vector.tensor_copy(accs[q_j][:], pair[:])
                else:
                    nc.vector.tensor_add(accs[q_j][:], accs[q_j][:], pair[:])
                if q_g == 7:
                    denom_tail(q_j)

    nc.finalize()
    return nc


class TileCtx:
    """with TileCtx(tile, nc) as (tc, ctx): ... -- TileContext + ExitStack."""

    def __init__(self, tile_mod, nc):
        self.tc = tile_mod.TileContext(nc)
        self.ctx = ExitStack()

    def __enter__(self):
        self.tc.__enter__()
        self.ctx.__enter__()
        return self.tc, self.ctx

    def __exit__(self, *exc):
        self.ctx.__exit__(*exc)
        return self.tc.__exit__(*exc)


def _run(x, Wq, bq, Wk, bk, Wv, bv, gamma, trace=False, tmpdir=None):
    from concourse.bass_utils import run_bass_kernel_spmd

    B = x.shape[0]
    g = float(np.asarray(gamma).reshape(-1)[0])

    f32 = np.float32
    wq4 = np.tile(np.asarray(Wq, dtype=f32).T, (1, 4))
    wk4 = np.tile(np.asarray(Wk, dtype=f32).T, (1, 4))
    wvt = (g * np.asarray(Wv, dtype=f32)).T
    wpack = np.ascontiguousarray(np.concatenate([wq4, wk4, wvt], axis=1))
    bq4 = np.tile(np.asarray(bq, dtype=f32), 4).reshape(128, 1)
    bk4 = np.tile(np.asarray(bk, dtype=f32), 4).reshape(128, 1)
    gbv = (g * np.asarray(bv, dtype=f32)).reshape(C, 1)
    bpack = np.ascontiguousarray(
        np.concatenate([bq4, bk4, gbv[0:128], gbv[128:256]], axis=1)
    )

    nc = _build_program()

    in_maps = []
    for b in range(B):
        in_maps.append(
            {
                "x": np.ascontiguousarray(np.asarray(x[b], dtype=f32).reshape(C, N)),
                "wpack": wpack,
                "bpack": bpack,
            }
        )
    res = run_bass_kernel_spmd(
        nc, in_maps, core_ids=list(range(B)), trace=trace, tmpdir=tmpdir
    )
    out = np.stack([res.results[b]["out"] for b in range(B)], axis=0)
    out = out.reshape(x.shape).astype(np.float32)
    return out, res


def kernel(x, Wq, bq, Wk, bk, Wv, bv, gamma):
    out, _ = _run(x, Wq, bq, Wk, bk, Wv, bv, gamma, trace=False)
    return out



# revision 10
# speedup vs baseline: 1.0370x; 1.0370x over previous
"""Trainium2 Bass kernel for AttentionConv2d (self-attention over 64x64 pixels).

Reference math (per image b):
    xf = x.reshape(C, N)                      # C=256, N=4096
    q  = Wq @ xf + bq                         # [32, N]
    k  = Wk @ xf + bk                         # [32, N]
    v  = Wv @ xf + bv                         # [256, N]
    corr[i, j] = sum_c q[c, i] * k[c, j]      # [N, N]
    beta = softmax(corr, axis=0)              # over i, per column j
    att[c, j] = gamma * sum_i v[c, i] * beta[i, j]
    out = att.reshape(C, H, W) + x

Sharding: data-parallel over batch, one image per NeuronCore (8 cores).

Steady state is Scalar-engine bound (128 exp ACTIVATEs x ~1.11us = 142.7us);
everything else is organized so the exp stream starts as early as possible
and never stalls:
  - input DMA: one ordered queue (sync engine), weights first, then x in 8
    single-transfer chunks (x is passed host-side as [128, 2, 4096] so each
    [256,512] chunk is one 3D-AP transfer).  Transfers stripe over all 16
    DMA engines and complete in order, so chunk c lands ~1.8us after c-1.
  - PE warmup: 9 N=512 dummy matmuls (~3.6us cold) timed so the HAM clock
    gate releases (1.2 -> 2.4 GHz) right as chunk 0 arrives; the chunk loop
    then runs warm and its 2.3us/chunk exp stream is the pacer.
  - q/k projection PSUMs live in the att-accumulator banks (idle until the
    first PV) so the eps pool only rotates corr quads + v^T tiles and the
    next chunk's corr never waits on a projection slot.
  - corr matmuls are 4x row-tiled (tile_position=(32r,0), K=32 strips); q/k
    are produced 4x-replicated by widening the projection weights host-side.
  - E is stored in per-quad bf16 tiles (not per-block): finer WAR granularity
    removes the exp stalls at block boundaries that a 3-deep block pool had.
  - j-blocks: 6x512 + 4x256.  The narrow tail blocks keep the exp width at
    [128,1024] (one ACTIVATE per quad) but halve the PV drain that trails
    the final exp: tail is ~5us instead of ~20us.
  - flat software pipeline over quads: PV lags corr/exp by LAG=8 quads, the
    denominator accumulation (bf16 adds on DVE) lags by DLAG=2 and is
    emitted after the att-release muls so those win the DVE queue and the
    next block's PV start never stalls >1us (a >3.4us PE gap would also
    re-throttle the HAM clock).
  - softmax denominator: per-quad accumulation on DVE, partition-reduce +
    broadcast on the otherwise-idle GpSimd engine (no PSUM slot); the last
    block uses a ones-matmul broadcast instead for a shorter tail.
gamma is folded into Wv host-side; gamma*bv is added at the end (softmax
weights sum to 1, so the v-bias is a per-channel constant).
"""

import sys

sys.path.insert(0, "/opt/trn_rl_repo")

from contextlib import ExitStack

import numpy as np

C = 256
CR = 32
N = 4096
CH = 128          # channel half (partition dim)
IT = 128          # i-tile height (partition dim of E tiles)
NC = 8            # x column chunks (512 wide)
NWIDE = 8         # 512-wide j-blocks
NNARROW = 0       # 256-wide j-blocks (disabled: narrow corr matmuls --
                  # two row-strips into one PSUM bank -- hang real HW)
LAG = 8           # initial quads of PV lag behind corr/exp
GAIN = 6          # quads of lag clawed back linearly over the run: the PE
                  # has ~2us/block of slack vs the ACT exp stream, so PV
                  # finishes ~2 quads after the last exp instead of 8,
                  # cutting the post-exp drain from ~17us to ~7us
DLAG = 1          # quads of denominator-accumulation lag behind exp
NQ = 8 * (NWIDE + NNARROW)   # 64 quads total

# (j0, width) per block; quad qi -> block qi//8, sub-quad g = qi%8,
# i-tiles 4g..4g+3
BLOCKS = [(512 * b, 512) for b in range(NWIDE)] + [
    (512 * NWIDE + 256 * b, 256) for b in range(NNARROW)
]


def _build_program():
    import concourse.bass as bass
    import concourse.mybir as mybir
    from concourse import bacc, bass_isa, tile

    f32 = mybir.dt.float32
    f32r = mybir.dt.float32r
    bf16 = mybir.dt.bfloat16
    EXP = mybir.ActivationFunctionType.Exp
    ADD = mybir.AluOpType.add
    ts = bass.ts

    nc = bacc.Bacc()
    # x host-packed as [128, 2, 4096]: [p, h, col] = xf[h*128+p, col] so one
    # 3D-AP transfer moves a full [256, 512] chunk into xf[:, c*1024:+1024]
    x_d = nc.declare_dram_parameter("x", [CH, 2, N], f32r, isOutput=False)
    # wpack host-packed as [128, 1024]: [p, h*512 + {0:128 wq4, 128:256 wk4,
    # 256:512 wvt}] (one transfer)
    wpack_d = nc.declare_dram_parameter("wpack", [CH, 1024], f32r, isOutput=False)
    bpack_d = nc.declare_dram_parameter("bpack", [128, 4], f32, isOutput=False)
    out_d = nc.declare_dram_parameter("out", [C, N], f32, isOutput=True)

    with TileCtx(tile, nc) as (tc, ctx):
        const = ctx.enter_context(tc.tile_pool(name="const", bufs=1))
        vtp = ctx.enter_context(tc.tile_pool(name="vtp", bufs=1))
        ebw = ctx.enter_context(tc.tile_pool(name="ebw", bufs=11))   # wide E quads
        ebn = ctx.enter_context(tc.tile_pool(name="ebn", bufs=11))   # narrow E quads
        pairp = ctx.enter_context(tc.tile_pool(name="pairp", bufs=2))
        accp = ctx.enter_context(tc.tile_pool(name="accp", bufs=2))
        rbp = ctx.enter_context(tc.tile_pool(name="rbp", bufs=2))
        scrp = ctx.enter_context(tc.tile_pool(name="scrp", bufs=2))
        outp = ctx.enter_context(tc.tile_pool(name="outp", bufs=4))
        # PSUM: eps 3x[128,1024] = 6 banks (corr quads, v^T) + 2x[128,512]
        # att accumulators = 8 banks.  att banks double as the q/k
        # projection PSUMs during the head (PV starts long after).
        eps_p = ctx.enter_context(tc.tile_pool(name="eps_p", bufs=3, space="PSUM"))
        att0_p = ctx.enter_context(tc.tile_pool(name="att0_p", bufs=1, space="PSUM"))
        att1_p = ctx.enter_context(tc.tile_pool(name="att1_p", bufs=1, space="PSUM"))

        # ---- resident inputs: one ordered DMA queue, weights first --------
        wtile = const.tile([CH, 1024], f32r, name="wtile")
        nc.sync.dma_start(out=wtile[:], in_=wpack_d[:, :])
        bpack = const.tile([128, 4], f32, name="bpack")
        nc.sync.dma_start(out=bpack[:], in_=bpack_d[:, :])
        xf = const.tile([CH, NC * 1024], f32r, name="xf")
        for c in range(NC):
            for h in range(2):
                nc.sync.dma_start(
                    out=xf[:, c * 1024 + h * 512:c * 1024 + (h + 1) * 512],
                    in_=x_d[:, h, ts(c, 512)],
                )

        def xcol(c, h, off=0):
            return c * 1024 + h * 512 + off

        wq4t = [wtile[:, h * 512 + 0:h * 512 + 128] for h in range(2)]
        wk4t = [wtile[:, h * 512 + 128:h * 512 + 256] for h in range(2)]
        wvt = [wtile[:, h * 512 + 256:h * 512 + 512] for h in range(2)]
        bq4_t = bpack[:, 0:1]
        bk4_t = bpack[:, 1:2]
        gbv = [bpack[:, 2 + h:3 + h] for h in range(2)]

        ones_b = const.tile([128, 128], bf16, name="ones_b")
        nc.vector.memset(ones_b[:], 1.0)
        ones512 = const.tile([128, 512], bf16, name="ones512")
        nc.vector.memset(ones512[:], 1.0)
        # PE warmup: ~3.6us of cold N=512 dummy matmuls so the HAM clock
        # gate releases right as chunk 0 lands; everything after runs at
        # 2.4 GHz.
        warm = eps_p.tile([128, 1024], f32, tag="eps", name="eps")
        for _ in range(9):
            nc.tensor.matmul(
                warm[:, 0:512], lhsT=ones_b[:], rhs=ones512[:], start=True, stop=True
            )

        q4 = const.tile([128, N], bf16, name="q4")
        k4 = const.tile([128, N], bf16, name="k4")
        vt = []
        equads = {}

        def corr_exp_quad(qi):
            """4x row-tiled corr matmuls + exp(s) for quad qi."""
            bi, g = divmod(qi, 8)
            j0, w = BLOCKS[bi]
            if w == 512:
                epsA = eps_p.tile([128, 1024], f32, tag="eps", name="eps")
                epsB = eps_p.tile([128, 1024], f32, tag="eps", name="eps")
                for r in range(4):
                    i = 4 * g + r
                    dst = epsA if r < 2 else epsB
                    nc.tensor.matmul(
                        dst[:, ts(r % 2, 512)],
                        lhsT=q4[32 * r:32 * (r + 1), ts(i, IT)],
                        rhs=k4[32 * r:32 * (r + 1), j0:j0 + 512],
                        start=True,
                        stop=True,
                        tile_position=(32 * r, 0),
                    )
                eq = ebw.tile([IT, 2048], bf16, tag="eq", name="eq")
                nc.scalar.activation(eq[:, 0:1024], epsA[:], EXP)
                nc.scalar.activation(eq[:, 1024:2048], epsB[:], EXP)
            else:
                eps1 = eps_p.tile([128, 1024], f32, tag="eps", name="eps")
                for r in range(4):
                    i = 4 * g + r
                    nc.tensor.matmul(
                        eps1[:, ts(r, 256)],
                        lhsT=q4[32 * r:32 * (r + 1), ts(i, IT)],
                        rhs=k4[32 * r:32 * (r + 1), j0:j0 + 256],
                        start=True,
                        stop=True,
                        tile_position=(32 * r, 0),
                    )
                eq = ebn.tile([IT, 1024], bf16, tag="eq", name="eq")
                nc.scalar.activation(eq[:], eps1[:], EXP)
            equads[qi] = eq

        # ---- head: x chunks -> projections, v^T, block-0 corr/exp --------
        for c in range(NC):
            csl = ts(c, 512)
            for (dst, wt, bias, pool) in (
                (q4, wq4t, bq4_t, att0_p),
                (k4, wk4t, bk4_t, att1_p),
            ):
                ps = pool.tile([CH, 512], f32, tag="ps", name="ps")
                for h in range(2):
                    nc.tensor.matmul(
                        ps[:],
                        lhsT=wt[h],
                        rhs=xf[:, xcol(c, h):xcol(c, h) + 512],
                        start=(h == 0),
                        stop=(h == 1),
                    )
                nc.vector.tensor_scalar_add(dst[:, csl], ps[:], bias)
            corr_exp_quad(c)
            psv = eps_p.tile([128, 1024], f32, tag="eps", name="eps")
            for t4 in range(4):
                for h in range(2):
                    nc.tensor.matmul(
                        psv[:, ts(t4, C)],
                        lhsT=xf[:, xcol(c, h, t4 * 128):xcol(c, h, t4 * 128) + 128],
                        rhs=wvt[h],
                        start=(h == 0),
                        stop=(h == 1),
                    )
            vtile = vtp.tile([128, 1024], bf16, name=f"vt{c}")
            nc.any.tensor_copy(vtile[:], psv[:])
            vt.append(vtile)

        # ---- denominator / PV / output helpers ----------------------------
        atts = {}
        rbs = {}
        accs = {}

        def den_step(qi):
            """Accumulate exp sums for quad qi (lags exp by DLAG quads)."""
            bi, g = divmod(qi, 8)
            j0, w = BLOCKS[bi]
            eq = equads[qi]
            if w == 512:
                pair = pairp.tile([128, 1024], bf16, tag="pair", name="pair")
                nc.vector.tensor_add(pair[:], eq[:, 0:1024], eq[:, 1024:2048])
                src = pair[:]
            else:
                src = eq[:]
            if g == 0:
                accs[bi] = accp.tile([128, 1024], bf16, tag="acc", name="acc")
                nc.vector.tensor_copy(accs[bi][:], src)
            else:
                nc.vector.tensor_add(accs[bi][:], accs[bi][:], src)
            if g == 7:
                den_tail(bi)

        def den_tail(bi):
            j0, w = BLOCKS[bi]
            acc = accs[bi]
            nc.vector.tensor_add(acc[:, 0:512], acc[:, 0:512], acc[:, 512:1024])
            if w == 256:
                nc.vector.tensor_add(acc[:, 0:256], acc[:, 0:256], acc[:, 256:512])
            s_part = acc[:, 0:w]
            rb = rbp.tile([CH, w], f32, tag="rb", name="rb")
            rscr = scrp.tile([CH, w], f32, tag="rscr", name="rscr")
            if w == 512 and bi < 5:
                # partition-reduce + broadcast on the (otherwise idle) GpSimd
                # engine: no PSUM slot, never blocks the corr quad pipeline
                s_bc = scrp.tile([CH, w], f32, tag="s_bc", name="s_bc")
                nc.gpsimd.partition_all_reduce(
                    s_bc[:], s_part, channels=CH, reduce_op=bass_isa.ReduceOp.add
                )
                nc.vector.reciprocal_approx_accurate(out=rb[:], in_=s_bc[:], scratch=rscr[:])
            else:
                # late blocks: ones-matmul broadcast -- lower latency than
                # the 3.8us gpsimd reduce, so rb is ready even though the
                # shrinking PV lag brings out_tail closer to the exps
                smm = eps_p.tile([128, 1024], f32, tag="eps", name="eps")
                nc.tensor.matmul(
                    smm[:, 0:w], lhsT=ones_b[:], rhs=s_part, start=True, stop=True
                )
                nc.vector.reciprocal_approx_accurate(out=rb[:], in_=smm[:, 0:w], scratch=rscr[:])
            rbs[bi] = rb

        def pv_quad(qi):
            """PV accumulation matmuls for quad qi (lags exp by LAG quads)."""
            bi, g = divmod(qi, 8)
            j0, w = BLOCKS[bi]
            if g == 0:
                atts[bi] = (
                    att0_p.tile([CH, w], f32, tag="ps", name="ps"),
                    att1_p.tile([CH, w], f32, tag="ps", name="ps"),
                )
            eq = equads[qi]
            for t4 in range(4):
                i = 4 * g + t4
                for h in range(2):
                    nc.tensor.matmul(
                        atts[bi][h][:],
                        lhsT=vt[g][:, t4 * C + h * CH:t4 * C + (h + 1) * CH],
                        rhs=eq[:, ts(t4, w)],
                        start=(i == 0),
                        stop=(i == 31),
                    )

        def out_tail(bi):
            j0, w = BLOCKS[bi]
            c, off = divmod(j0, 512)  # xf chunk/offset for the residual
            # both att-releasing muls first (frees both att banks ASAP for
            # the next block's PV), then the residual adds and DMAs
            os = []
            for h in range(2):
                o = outp.tile([CH, w], f32, tag="o", name="o")
                nc.vector.tensor_mul(o[:], atts[bi][h][:], rbs[bi][:])
                os.append(o)
            for h in range(2):
                xsl = xf[:, xcol(c, h, off):xcol(c, h, off) + w]
                nc.vector.scalar_tensor_tensor(
                    out=os[h][:],
                    in0=os[h][:],
                    scalar=gbv[h],
                    in1=xsl.bitcast(f32),
                    op0=ADD,
                    op1=ADD,
                )
                nc.sync.dma_start(out=out_d[h * CH:(h + 1) * CH, j0:j0 + w], in_=os[h][:])

        # ---- main flat pipeline over quads --------------------------------
        # exp side leads; PV trails with a lag that shrinks from LAG to
        # LAG-GAIN over the run (the PE's per-block slack absorbs the extra
        # quads), so the post-exp PV drain is ~2 quads, not 8.  Denominator
        # accumulation lags by DLAG and is emitted after out_tail's muls so
        # those win the strict-FIFO DVE queue.
        pv_cursor = 0

        def pv_advance(target):
            nonlocal pv_cursor
            while pv_cursor < min(target, NQ):
                pv_quad(pv_cursor)
                if pv_cursor % 8 == 7:
                    out_tail(pv_cursor // 8)
                pv_cursor += 1

        for k in range(DLAG, NQ):
            if 8 <= k:
                corr_exp_quad(k)
            pv_advance(k - LAG + (GAIN * max(0, k - 8)) // (NQ - 8) + 1)
            dq = k - DLAG
            if 0 <= dq < NQ:
                den_step(dq)
        for dq in range(NQ - DLAG, NQ):
            den_step(dq)
        pv_advance(NQ)

    nc.finalize()
    return nc


class TileCtx:
    """with TileCtx(tile, nc) as (tc, ctx): ... -- TileContext + ExitStack."""

    def __init__(self, tile_mod, nc):
        self.tc = tile_mod.TileContext(nc)
        self.ctx = ExitStack()

    def __enter__(self):
        self.tc.__enter__()
        self.ctx.__enter__()
        return self.tc, self.ctx

    def __exit__(self, *exc):
        self.ctx.__exit__(*exc)
        return self.tc.__exit__(*exc)


def _run(x, Wq, bq, Wk, bk, Wv, bv, gamma, trace=False, tmpdir=None):
    from concourse.bass_utils import run_bass_kernel_spmd

    B = x.shape[0]
    g = float(np.asarray(gamma).reshape(-1)[0])

    f32 = np.float32
    wq4 = np.tile(np.asarray(Wq, dtype=f32).T, (1, 4))      # [256, 128]
    wk4 = np.tile(np.asarray(Wk, dtype=f32).T, (1, 4))      # [256, 128]
    wvt = (g * np.asarray(Wv, dtype=f32)).T                  # [256, 256]
    wpack = np.concatenate([wq4, wk4, wvt], axis=1)          # [256, 512]
    # -> [128, 1024]: [p, h*512 + col]
    wpack = np.ascontiguousarray(
        wpack.reshape(2, 128, 512).transpose(1, 0, 2).reshape(128, 1024)
    )
    bq4 = np.tile(np.asarray(bq, dtype=f32), 4).reshape(128, 1)
    bk4 = np.tile(np.asarray(bk, dtype=f32), 4).reshape(128, 1)
    gbv = (g * np.asarray(bv, dtype=f32)).reshape(C, 1)
    bpack = np.ascontiguousarray(
        np.concatenate([bq4, bk4, gbv[0:128], gbv[128:256]], axis=1)
    )

    nc = _build_program()

    in_maps = []
    for b in range(B):
        xb = np.asarray(x[b], dtype=f32).reshape(C, N)
        # [128, 2, 4096]: [p, h, col] = xb[h*128+p, col]
        xb = np.ascontiguousarray(xb.reshape(2, 128, N).transpose(1, 0, 2))
        in_maps.append({"x": xb, "wpack": wpack, "bpack": bpack})
    res = run_bass_kernel_spmd(
        nc, in_maps, core_ids=list(range(B)), trace=trace, tmpdir=tmpdir
    )
    out = np.stack([res.results[b]["out"] for b in range(B)], axis=0)
    out = out.reshape(x.shape).astype(np.float32)
    return out, res


def kernel(x, Wq, bq, Wk, bk, Wv, bv, gamma):
    out, _ = _run(x, Wq, bq, Wk, bk, Wv, bv, gamma, trace=False)
    return out


# revision 16
# speedup vs baseline: 1.0395x; 1.0024x over previous
"""Trainium2 Bass kernel for AttentionConv2d (self-attention over 64x64 pixels).

Reference math (per image b):
    xf = x.reshape(C, N)                      # C=256, N=4096
    q  = Wq @ xf + bq                         # [32, N]
    k  = Wk @ xf + bk                         # [32, N]
    v  = Wv @ xf + bv                         # [256, N]
    corr[i, j] = sum_c q[c, i] * k[c, j]      # [N, N]
    beta = softmax(corr, axis=0)              # over i, per column j
    att[c, j] = gamma * sum_i v[c, i] * beta[i, j]
    out = att.reshape(C, H, W) + x

Sharding: data-parallel over batch, one image per NeuronCore (8 cores).

Steady state is Scalar-engine bound (128 exp ACTIVATEs x ~1.11us = 142.7us);
everything else is organized so the exp stream starts as early as possible
and never stalls:
  - input DMA: one ordered queue (sync engine), weights first, then x in 8
    single-transfer chunks (x is passed host-side as [128, 2, 4096] so each
    [256,512] chunk is one 3D-AP transfer).  Transfers stripe over all 16
    DMA engines and complete in order, so chunk c lands ~1.8us after c-1.
  - PE warmup: 9 N=512 dummy matmuls (~3.6us cold) timed so the HAM clock
    gate releases (1.2 -> 2.4 GHz) right as chunk 0 arrives; the chunk loop
    then runs warm and its 2.3us/chunk exp stream is the pacer.
  - q/k projection PSUMs live in the att-accumulator banks (idle until the
    first PV) so the eps pool only rotates corr quads + v^T tiles and the
    next chunk's corr never waits on a projection slot.
  - corr matmuls are 4x row-tiled (tile_position=(32r,0), K=32 strips); q/k
    are produced 4x-replicated by widening the projection weights host-side.
  - E is stored in per-quad bf16 tiles (not per-block): finer WAR granularity
    removes the exp stalls at block boundaries that a 3-deep block pool had.
  - j-blocks: 6x512 + 4x256.  The narrow tail blocks keep the exp width at
    [128,1024] (one ACTIVATE per quad) but halve the PV drain that trails
    the final exp: tail is ~5us instead of ~20us.
  - flat software pipeline over quads: PV lags corr/exp by LAG=8 quads, the
    denominator accumulation (bf16 adds on DVE) lags by DLAG=2 and is
    emitted after the att-release muls so those win the DVE queue and the
    next block's PV start never stalls >1us (a >3.4us PE gap would also
    re-throttle the HAM clock).
  - softmax denominator: per-quad accumulation on DVE, partition-reduce +
    broadcast on the otherwise-idle GpSimd engine (no PSUM slot); the last
    block uses a ones-matmul broadcast instead for a shorter tail.
gamma is folded into Wv host-side; gamma*bv is added at the end (softmax
weights sum to 1, so the v-bias is a per-channel constant).
"""

import sys

sys.path.insert(0, "/opt/trn_rl_repo")

from contextlib import ExitStack

import numpy as np

C = 256
CR = 32
N = 4096
CH = 128          # channel half (partition dim)
IT = 128          # i-tile height (partition dim of E tiles)
NC = 8            # x column chunks (512 wide)
NWIDE = 8         # 512-wide j-blocks
NNARROW = 0       # 256-wide j-blocks (disabled: narrow corr matmuls --
                  # two row-strips into one PSUM bank -- hang real HW)
LAG = 8           # initial quads of PV lag behind corr/exp
GAIN = 6          # quads of lag clawed back linearly over the run: the PE
                  # has ~2us/block of slack vs the ACT exp stream, so PV
                  # finishes ~2 quads after the last exp instead of 8,
                  # cutting the post-exp drain from ~17us to ~7us
DLAG = 1          # quads of denominator-accumulation lag behind exp
NQ = 8 * (NWIDE + NNARROW)   # 64 quads total

# (j0, width) per block; quad qi -> block qi//8, sub-quad g = qi%8,
# i-tiles 4g..4g+3
BLOCKS = [(512 * b, 512) for b in range(NWIDE)] + [
    (512 * NWIDE + 256 * b, 256) for b in range(NNARROW)
]


def _build_program():
    import concourse.bass as bass
    import concourse.mybir as mybir
    from concourse import bacc, bass_isa, tile

    f32 = mybir.dt.float32
    f32r = mybir.dt.float32r
    bf16 = mybir.dt.bfloat16
    EXP = mybir.ActivationFunctionType.Exp
    ADD = mybir.AluOpType.add
    ts = bass.ts

    nc = bacc.Bacc()
    # x host-packed as [128, 2, 4096]: [p, h, col] = xf[h*128+p, col] so one
    # 3D-AP transfer moves a full [256, 512] chunk into xf[:, c*1024:+1024]
    x_d = nc.declare_dram_parameter("x", [CH, 2, N], f32r, isOutput=False)
    # wpack host-packed as [128, 1024]: [p, h*512 + {0:128 wq4, 128:256 wk4,
    # 256:512 wvt}] (one transfer)
    wpack_d = nc.declare_dram_parameter("wpack", [CH, 1024], f32r, isOutput=False)
    bpack_d = nc.declare_dram_parameter("bpack", [128, 4], f32, isOutput=False)
    out_d = nc.declare_dram_parameter("out", [C, N], f32, isOutput=True)

    with TileCtx(tile, nc) as (tc, ctx):
        const = ctx.enter_context(tc.tile_pool(name="const", bufs=1))
        vtp = ctx.enter_context(tc.tile_pool(name="vtp", bufs=1))
        ebw = ctx.enter_context(tc.tile_pool(name="ebw", bufs=13))   # wide E quads
        ebn = ctx.enter_context(tc.tile_pool(name="ebn", bufs=11))   # narrow E quads
        pairp = ctx.enter_context(tc.tile_pool(name="pairp", bufs=2))
        accp = ctx.enter_context(tc.tile_pool(name="accp", bufs=2))
        rbp = ctx.enter_context(tc.tile_pool(name="rbp", bufs=2))
        scrp = ctx.enter_context(tc.tile_pool(name="scrp", bufs=2))
        outp = ctx.enter_context(tc.tile_pool(name="outp", bufs=4))
        # PSUM: eps 3x[128,1024] = 6 banks (corr quads, v^T) + 2x[128,512]
        # att accumulators = 8 banks.  att banks double as the q/k
        # projection PSUMs during the head (PV starts long after).
        eps_p = ctx.enter_context(tc.tile_pool(name="eps_p", bufs=3, space="PSUM"))
        att0_p = ctx.enter_context(tc.tile_pool(name="att0_p", bufs=1, space="PSUM"))
        att1_p = ctx.enter_context(tc.tile_pool(name="att1_p", bufs=1, space="PSUM"))

        # ---- resident inputs --------------------------------------------
        # weights on the scalar queue (parallel with x), x chunks in order
        # on the sync queue so chunk c completes ~1.7us after chunk c-1
        wtile = const.tile([CH, 1024], f32r, name="wtile")
        nc.scalar.dma_start(out=wtile[:], in_=wpack_d[:, :])
        bpack = const.tile([128, 4], f32, name="bpack")
        nc.scalar.dma_start(out=bpack[:], in_=bpack_d[:, :])
        xf = const.tile([CH, NC * 1024], f32r, name="xf")
        for c in range(NC):
            for h in range(2):
                nc.sync.dma_start(
                    out=xf[:, c * 1024 + h * 512:c * 1024 + (h + 1) * 512],
                    in_=x_d[:, h, ts(c, 512)],
                )

        def xcol(c, h, off=0):
            return c * 1024 + h * 512 + off

        wq4t = [wtile[:, h * 512 + 0:h * 512 + 128] for h in range(2)]
        wk4t = [wtile[:, h * 512 + 128:h * 512 + 256] for h in range(2)]
        wvt = [wtile[:, h * 512 + 256:h * 512 + 512] for h in range(2)]
        bq4_t = bpack[:, 0:1]
        bk4_t = bpack[:, 1:2]
        gbv = [bpack[:, 2 + h:3 + h] for h in range(2)]

        ones512 = const.tile([128, 512], bf16, name="ones512")
        nc.vector.memset(ones512[:], 1.0)
        ones_b = const.tile([128, 128], bf16, name="ones_b")
        nc.vector.memset(ones_b[:], 1.0)
        # PE warmup: ~5us of cold N=512 dummy matmuls bridging the gap
        # until chunk 0 lands, so the HAM clock gate sees one fully-busy
        # 3.4us window and releases (1.2 -> 2.4 GHz) before the first
        # projection; a PE idle hole here would restart the wait.
        warm = eps_p.tile([128, 1024], f32, tag="eps", name="eps")
        for _ in range(12):
            nc.tensor.matmul(
                warm[:, 0:512], lhsT=ones_b[:], rhs=ones512[:], start=True, stop=True
            )

        q4 = const.tile([128, N], bf16, name="q4")
        k4 = const.tile([128, N], bf16, name="k4")
        vt = []
        equads = {}

        def corr_exp_quad(qi):
            """4x row-tiled corr matmuls + exp(s) for quad qi."""
            bi, g = divmod(qi, 8)
            j0, w = BLOCKS[bi]
            if w == 512:
                epsA = eps_p.tile([128, 1024], f32, tag="eps", name="eps")
                epsB = eps_p.tile([128, 1024], f32, tag="eps", name="eps")
                for r in range(4):
                    i = 4 * g + r
                    dst = epsA if r < 2 else epsB
                    nc.tensor.matmul(
                        dst[:, ts(r % 2, 512)],
                        lhsT=q4[32 * r:32 * (r + 1), ts(i, IT)],
                        rhs=k4[32 * r:32 * (r + 1), j0:j0 + 512],
                        start=True,
                        stop=True,
                        tile_position=(32 * r, 0),
                    )
                eq = ebw.tile([IT, 2048], bf16, tag="eq", name="eq")
                nc.scalar.activation(eq[:, 0:1024], epsA[:], EXP)
                nc.scalar.activation(eq[:, 1024:2048], epsB[:], EXP)
            else:
                eps1 = eps_p.tile([128, 1024], f32, tag="eps", name="eps")
                for r in range(4):
                    i = 4 * g + r
                    nc.tensor.matmul(
                        eps1[:, ts(r, 256)],
                        lhsT=q4[32 * r:32 * (r + 1), ts(i, IT)],
                        rhs=k4[32 * r:32 * (r + 1), j0:j0 + 256],
                        start=True,
                        stop=True,
                        tile_position=(32 * r, 0),
                    )
                eq = ebn.tile([IT, 1024], bf16, tag="eq", name="eq")
                nc.scalar.activation(eq[:], eps1[:], EXP)
            equads[qi] = eq

        # ---- head: x chunks -> projections, v^T, block-0 corr/exp --------
        for c in range(NC):
            csl = ts(c, 512)
            for (dst, wt, bias, pool) in (
                (q4, wq4t, bq4_t, att0_p),
                (k4, wk4t, bk4_t, att1_p),
            ):
                ps = pool.tile([CH, 512], f32, tag="ps", name="ps")
                for h in range(2):
                    nc.tensor.matmul(
                        ps[:],
                        lhsT=wt[h],
                        rhs=xf[:, xcol(c, h):xcol(c, h) + 512],
                        start=(h == 0),
                        stop=(h == 1),
                    )
                nc.vector.tensor_scalar_add(dst[:, csl], ps[:], bias)
            corr_exp_quad(c)
            psv = eps_p.tile([128, 1024], f32, tag="eps", name="eps")
            for t4 in range(4):
                for h in range(2):
                    nc.tensor.matmul(
                        psv[:, ts(t4, C)],
                        lhsT=xf[:, xcol(c, h, t4 * 128):xcol(c, h, t4 * 128) + 128],
                        rhs=wvt[h],
                        start=(h == 0),
                        stop=(h == 1),
                    )
            vtile = vtp.tile([128, 1024], bf16, name=f"vt{c}")
            # force DVE: nc.any let the scheduler put half of these on the
            # Scalar engine, delaying the exp stream
            nc.vector.tensor_copy(vtile[:], psv[:])
            vt.append(vtile)

        # ---- denominator / PV / output helpers ----------------------------
        atts = {}
        rbs = {}
        accs = {}

        def den_step(qi):
            """Accumulate exp sums for quad qi (lags exp by DLAG quads)."""
            bi, g = divmod(qi, 8)
            j0, w = BLOCKS[bi]
            eq = equads[qi]
            if w == 512:
                pair = pairp.tile([128, 1024], bf16, tag="pair", name="pair")
                nc.vector.tensor_add(pair[:], eq[:, 0:1024], eq[:, 1024:2048])
                src = pair[:]
            else:
                src = eq[:]
            if g == 0:
                accs[bi] = accp.tile([128, 1024], bf16, tag="acc", name="acc")
                nc.vector.tensor_copy(accs[bi][:], src)
            else:
                nc.vector.tensor_add(accs[bi][:], accs[bi][:], src)
            if g == 7:
                den_tail(bi)

        def den_tail(bi):
            j0, w = BLOCKS[bi]
            acc = accs[bi]
            nc.vector.tensor_add(acc[:, 0:512], acc[:, 0:512], acc[:, 512:1024])
            if w == 256:
                nc.vector.tensor_add(acc[:, 0:256], acc[:, 0:256], acc[:, 256:512])
            s_part = acc[:, 0:w]
            rb = rbp.tile([CH, w], f32, tag="rb", name="rb")
            rscr = scrp.tile([CH, w], f32, tag="rscr", name="rscr")
            # partition-reduce + broadcast via a ones-matmul.  A gpsimd
            # partition_all_reduce takes 3.8us, and the DVE reciprocal
            # waiting on it head-blocks the strict-FIFO DVE queue --
            # delaying denominator adds and att-release muls and stalling
            # both the exp stream and the next block's PV.
            smm = eps_p.tile([128, 1024], f32, tag="eps", name="eps")
            nc.tensor.matmul(
                smm[:, 0:w], lhsT=ones_b[:], rhs=s_part, start=True, stop=True
            )
            nc.vector.reciprocal_approx_accurate(out=rb[:], in_=smm[:, 0:w], scratch=rscr[:])
            rbs[bi] = rb

        def pv_quad(qi):
            """PV accumulation matmuls for quad qi (lags exp by LAG quads)."""
            bi, g = divmod(qi, 8)
            j0, w = BLOCKS[bi]
            if g == 0:
                atts[bi] = (
                    att0_p.tile([CH, w], f32, tag="ps", name="ps"),
                    att1_p.tile([CH, w], f32, tag="ps", name="ps"),
                )
            eq = equads[qi]
            for t4 in range(4):
                i = 4 * g + t4
                for h in range(2):
                    nc.tensor.matmul(
                        atts[bi][h][:],
                        lhsT=vt[g][:, t4 * C + h * CH:t4 * C + (h + 1) * CH],
                        rhs=eq[:, ts(t4, w)],
                        start=(i == 0),
                        stop=(i == 31),
                    )

        def out_tail(bi):
            j0, w = BLOCKS[bi]
            c, off = divmod(j0, 512)  # xf chunk/offset for the residual
            last = bi == len(BLOCKS) - 1

            def _mul(h):
                o = outp.tile([CH, w], f32, tag="o", name="o")
                nc.vector.tensor_mul(o[:], atts[bi][h][:], rbs[bi][:])
                return o

            def _sttdma(h, o):
                xsl = xf[:, xcol(c, h, off):xcol(c, h, off) + w]
                nc.vector.scalar_tensor_tensor(
                    out=o[:], in0=o[:], scalar=gbv[h], in1=xsl.bitcast(f32),
                    op0=ADD, op1=ADD,
                )
                # last block: h1 issue on the (now idle) scalar queue so the
                # two output transfers pipeline
                eng = nc.scalar if (last and h == 1) else nc.sync
                eng.dma_start(out=out_d[h * CH:(h + 1) * CH, j0:j0 + w], in_=o[:])

            if last:
                # drain order: get h0's output DMA moving while h1 wraps up
                o0 = _mul(0)
                _sttdma(0, o0)
                o1 = _mul(1)
                _sttdma(1, o1)
            else:
                # both att-releasing muls first (frees both att banks ASAP
                # for the next block's PV), then the residual adds and DMAs
                os = [_mul(0), _mul(1)]
                _sttdma(0, os[0])
                _sttdma(1, os[1])

        # ---- main flat pipeline over quads --------------------------------
        # exp side leads; PV trails with a lag that shrinks from LAG to
        # LAG-GAIN over the run (the PE's per-block slack absorbs the extra
        # quads), so the post-exp PV drain is ~2 quads, not 8.  Denominator
        # accumulation lags by DLAG and is emitted after out_tail's muls so
        # those win the strict-FIFO DVE queue.
        pv_cursor = 0

        def pv_advance(target):
            nonlocal pv_cursor
            while pv_cursor < min(target, NQ):
                pv_quad(pv_cursor)
                if pv_cursor % 8 == 7:
                    out_tail(pv_cursor // 8)
                pv_cursor += 1

        for k in range(DLAG, NQ):
            if 8 <= k:
                corr_exp_quad(k)
            pv_advance(k - LAG + (GAIN * max(0, k - 8)) // (NQ - 8) + 1)
            dq = k - DLAG
            if 0 <= dq < NQ:
                den_step(dq)
        for dq in range(NQ - DLAG, NQ):
            den_step(dq)
        pv_advance(NQ)

    nc.finalize()
    return nc


class TileCtx:
    """with TileCtx(tile, nc) as (tc, ctx): ... -- TileContext + ExitStack."""

    def __init__(self, tile_mod, nc):
        self.tc = tile_mod.TileContext(nc)
        self.ctx = ExitStack()

    def __enter__(self):
        self.tc.__enter__()
        self.ctx.__enter__()
        return self.tc, self.ctx

    def __exit__(self, *exc):
        self.ctx.__exit__(*exc)
        return self.tc.__exit__(*exc)


def _run(x, Wq, bq, Wk, bk, Wv, bv, gamma, trace=False, tmpdir=None):
    from concourse.bass_utils import run_bass_kernel_spmd

    B = x.shape[0]
    g = float(np.asarray(gamma).reshape(-1)[0])

    f32 = np.float32
    wq4 = np.tile(np.asarray(Wq, dtype=f32).T, (1, 4))      # [256, 128]
    wk4 = np.tile(np.asarray(Wk, dtype=f32).T, (1, 4))      # [256, 128]
    wvt = (g * np.asarray(Wv, dtype=f32)).T                  # [256, 256]
    wpack = np.concatenate([wq4, wk4, wvt], axis=1)          # [256, 512]
    # -> [128, 1024]: [p, h*512 + col]
    wpack = np.ascontiguousarray(
        wpack.reshape(2, 128, 512).transpose(1, 0, 2).reshape(128, 1024)
    )
    bq4 = np.tile(np.asarray(bq, dtype=f32), 4).reshape(128, 1)
    bk4 = np.tile(np.asarray(bk, dtype=f32), 4).reshape(128, 1)
    gbv = (g * np.asarray(bv, dtype=f32)).reshape(C, 1)
    bpack = np.ascontiguousarray(
        np.concatenate([bq4, bk4, gbv[0:128], gbv[128:256]], axis=1)
    )

    nc = _build_program()

    in_maps = []
    for b in range(B):
        xb = np.asarray(x[b], dtype=f32).reshape(C, N)
        # [128, 2, 4096]: [p, h, col] = xb[h*128+p, col]
        xb = np.ascontiguousarray(xb.reshape(2, 128, N).transpose(1, 0, 2))
        in_maps.append({"x": xb, "wpack": wpack, "bpack": bpack})
    res = run_bass_kernel_spmd(
        nc, in_maps, core_ids=list(range(B)), trace=trace, tmpdir=tmpdir
    )
    out = np.stack([res.results[b]["out"] for b in range(B)], axis=0)
    out = out.reshape(x.shape).astype(np.float32)
    return out, res


def kernel(x, Wq, bq, Wk, bk, Wv, bv, gamma):
    out, _ = _run(x, Wq, bq, Wk, bk, Wv, bv, gamma, trace=False)
    return out
